# revision 1
# baseline (speedup 1.0000x reference)
"""Trainium2 Bass kernel for nn_DiffusionBlock (anisotropic diffusion step).

Sharding: pure data-parallel over batch. 16 batches -> 8 cores x 2 batches;
each core processes 4 images (2 batches x 2 channels) of 768x768.

Math (validated vs reference to 9e-8 rel in fp32; see kernel_v2_f32.py):
  grid 769x769 (i,j in 0..768), pu = edge-padded u (clamp at row/col 767)
  g1 = P11-P00 ; g2 = P01-P10 ; m = P01+P10-P00-P11 ; gp = g1+g2 ; gm = g1-g2
  with k4 = tau/8 folded into the a/b/c downcasts (Ab = k4*a etc., fp16):
    s12 = Ab*gp + Bb*gm ; s34 = Bb*gp + Cb*gm
    d12 = (Ab - k4|b|)*m ; d34 = (Cb - k4|b|)*m    ((1-2a) lives in PE weights)
  out[p] = u[p] + P[p+1] + Q[p] computed on PE as 8 accumulating matmuls:
    acc = W1@s12_> - W1@s12 + W2@(s34_> + s34) + W2d@(d12_> + d34_>)
          - W2d@(d12 + d34)
    W1 = S+I, W2 = S-I, W2d = (1-2a)(S-I), S = subdiagonal row-shift matrix
  then out = acc + u (one DVE op), stored fp16 [H, NIMG, W] (host reassembles).

Engine split per 128-row x 2-image group: DMA-sync 5 spread loads; ACT downcasts
(+k4 scaling) and |b| via Square/Sqrt; DVE 16 wide fp16 ops (2x mode via even
widths); PE 32 matmuls; GPSIMD SWDGE output stores (spread across SDMA engines).
Row tiling: out rows [t0, t0+126], t0 in {0,127,...,635,640}; row 767 from a
small tail pass (bottom-edge identities zero the d-terms there).
"""

import numpy as np
import ml_dtypes
from contextlib import ExitStack

import concourse.bass as bass
import concourse.mybir as mybir
import concourse.tile as tile
from concourse.bacc import Bacc
from concourse.bass_utils import run_bass_kernel_spmd

F32 = mybir.dt.float32
F16 = mybir.dt.float16
OP = mybir.AluOpType
AF = mybir.ActivationFunctionType

B, C, H, W = 16, 2, 768, 768
NCORES = 8
NIMG = 4          # images per core
IMGG = 2          # images per tile-group
GW = 770          # padded width of pu tiles (f32)
PW = 772          # fp16 pu tiles: even width + finite junk guard cols
T0S = [0, 127, 254, 381, 508, 635, 640]


def _build(k4, k4m):
    nc = Bacc()
    u_d = nc.declare_dram_parameter("u", [NIMG, H, W], F32, isOutput=False)
    a_d = nc.declare_dram_parameter("a", [NIMG, H + 2, W + 2], F32, isOutput=False)
    b_d = nc.declare_dram_parameter("b", [NIMG, H + 2, W + 2], F32, isOutput=False)
    c_d = nc.declare_dram_parameter("c", [NIMG, H + 2, W + 2], F32, isOutput=False)
    s_d = nc.declare_dram_parameter("smat", [6, 128, 128], F16, isOutput=False)
    o_d = nc.declare_dram_parameter("out", [H, NIMG, W], F16, isOutput=True)

    with tile.TileContext(nc) as tc, ExitStack() as ctx:
        consts = ctx.enter_context(tc.tile_pool(name="consts", bufs=1))
        loads = ctx.enter_context(tc.tile_pool(name="loads", bufs=2))
        scr = ctx.enter_context(tc.tile_pool(name="scr", bufs=2))
        outp = ctx.enter_context(tc.tile_pool(name="outp", bufs=2))
        psum = ctx.enter_context(
            tc.tile_pool(name="psum", bufs=2, space=bass.MemorySpace.PSUM))

        Wm = []
        for wi in range(6):
            wt = consts.tile([128, 128], F16, tag=f"w{wi}", name=f"w{wi}")
            nc.sync.dma_start(out=wt[:], in_=s_d[wi])
            Wm.append(wt[:])
        W1, W1n, W2, W2n, W2d, W2dn = Wm

        def S(tag, w=GW, dt=F16):
            return scr.tile([128, IMGG, w], dt, tag=tag, name=tag)

        V = nc.vector
        GP = nc.gpsimd
        SC = nc.scalar

        for t0 in T0S:
            last = t0 == 640
            for g in range(NIMG // IMGG):
                ig0 = g * IMGG
                # ---- loads: one DMA descriptor per tile (HWDGE spreads) ----
                PU = loads.tile([128, IMGG, GW], F32, tag="pu")
                PU2 = loads.tile([128, IMGG, GW], F32, tag="pu2")
                nd2 = min(128, H - (t0 + 1))  # 128 except last tile (127)
                src = u_d[ig0:ig0 + IMGG, t0:t0 + 128, :]
                nc.sync.dma_start(out=PU[:, :, 0:W], in_=src.rearrange("i r c -> r i c"))
                src2 = u_d[ig0:ig0 + IMGG, t0 + 1:t0 + 1 + nd2, :]
                nc.sync.dma_start(out=PU2[0:nd2, :, 0:W], in_=src2.rearrange("i r c -> r i c"))
                if nd2 < 128:
                    srcc = u_d[ig0:ig0 + IMGG, H - 1:H, :]
                    nc.sync.dma_start(out=PU2[nd2:128, :, 0:W], in_=srcc.rearrange("i r c -> r i c"))
                SC.copy(PU[:, :, W:W + 2], PU[:, :, W - 1:W].to_broadcast([128, IMGG, 2]))
                SC.copy(PU2[:, :, W:W + 2], PU2[:, :, W - 1:W].to_broadcast([128, IMGG, 2]))
                A = loads.tile([128, IMGG, 769], F32, tag="a")
                Bt = loads.tile([128, IMGG, 769], F32, tag="b")
                Ct = loads.tile([128, IMGG, 769], F32, tag="c")
                for dram, buf in ((a_d, A), (b_d, Bt), (c_d, Ct)):
                    srcw = dram[ig0:ig0 + IMGG, 1 + t0:1 + t0 + 128, 1:W + 2]
                    nc.sync.dma_start(out=buf[:], in_=srcw.rearrange("i r c -> r i c"))

                # ---- ACT: fp16 downcasts (k4 folded into a,b,c); |b| ----
                PUb = loads.tile([128, IMGG, PW], F16, tag="pub")
                SC.copy(PUb[:, :, 0:GW], PU[:])
                SC.copy(PUb[:, :, GW:PW], PUb[:, :, GW - 1:GW].to_broadcast([128, IMGG, 2]))
                PU2b = loads.tile([128, IMGG, PW], F16, tag="pu2b")
                SC.copy(PU2b[:, :, 0:GW], PU2[:])
                SC.copy(PU2b[:, :, GW:PW], PU2b[:, :, GW - 1:GW].to_broadcast([128, IMGG, 2]))
                Ab = loads.tile([128, IMGG, GW], F16, tag="ab")
                SC.mul(Ab[:, :, 0:769], A[:], k4)
                Bb = loads.tile([128, IMGG, GW], F16, tag="bb")
                SC.mul(Bb[:, :, 0:769], Bt[:], k4)
                Cb = loads.tile([128, IMGG, GW], F16, tag="cb")
                SC.mul(Cb[:, :, 0:769], Ct[:], k4)
                bsq = S("bsq", 769, F32)
                SC.activation(bsq[:], Bt[:], AF.Square)
                absB = S("absb", GW)
                SC.activation(absB[:, :, 0:769], bsq[:], AF.Sqrt, scale=k4 * k4)
                # finite guard cols (junk col 769 flows through products only)
                GP.memset(Ab[:, :, 769:GW], 0.0)
                GP.memset(Bb[:, :, 769:GW], 0.0)
                GP.memset(Cb[:, :, 769:GW], 0.0)
                GP.memset(absB[:, :, 769:GW], 0.0)

                # ---- DVE stage A (fp16, even widths -> 2x mode) ----
                E = S("e", PW)
                V.tensor_sub(E[:], PU2b[:], PUb[:])
                g1 = S("g1", GW)
                V.tensor_sub(g1[:], PU2b[:, :, 1:GW + 1], PUb[:, :, 0:GW])
                g2 = S("g2", GW)
                V.tensor_sub(g2[:], PUb[:, :, 1:GW + 1], PU2b[:, :, 0:GW])
                gp = S("gp", GW)
                V.tensor_add(gp[:], g1[:], g2[:])
                gm = S("gm", GW)
                V.tensor_sub(gm[:], g1[:], g2[:])
                m = S("m", GW)
                V.tensor_sub(m[:], E[:, :, 0:GW], E[:, :, 1:GW + 1])
                am = S("am", GW)
                V.tensor_sub(am[:], Ab[:], absB[:])
                cm = S("cm", GW)
                V.tensor_sub(cm[:], Cb[:], absB[:])

                # ---- products (fp16 TT, 2x) ----
                t1 = S("t1", GW)
                V.tensor_mul(t1[:], Ab[:], gp[:])
                t2 = S("t2", GW)
                V.tensor_mul(t2[:], Bb[:], gm[:])
                s12 = S("s12", GW)
                V.tensor_add(s12[:], t1[:], t2[:])
                t3 = S("t1", GW)
                V.tensor_mul(t3[:], Bb[:], gp[:])
                t4 = S("t2", GW)
                V.tensor_mul(t4[:], Cb[:], gm[:])
                s34 = S("s34", GW)
                V.tensor_add(s34[:], t3[:], t4[:])
                d12 = S("d12", GW)
                V.tensor_mul(d12[:], am[:], m[:])
                d34 = S("d34", GW)
                V.tensor_mul(d34[:], cm[:], m[:])

                # ---- column stage + row shift fused on PE ----
                acc = psum.tile([128, IMGG, 1024], F32, tag="acc")
                for i in range(IMGG):
                    for c0, cw in ((0, 512), (512, 256)):
                        terms = (
                            (W1, s12, 1), (W1n, s12, 0),
                            (W2, s34, 1), (W2, s34, 0),
                            (W2d, d12, 1), (W2d, d34, 1),
                            (W2dn, d12, 0), (W2dn, d34, 0),
                        )
                        for ti, (wm, arr, sh) in enumerate(terms):
                            nc.tensor.matmul(
                                acc[:, i, c0:c0 + cw], wm,
                                arr[:, i, c0 + sh:c0 + sh + cw],
                                start=(ti == 0), stop=(ti == len(terms) - 1))

                ot = outp.tile([128, IMGG, W], F16, tag="ot")
                V.tensor_add(ot[0:127], acc[0:127, :, 0:W], PU[0:127, :, 0:W])

                # store out rows [t0, t0+126] (last tile: only rows 762..766)
                if not last:
                    p0, p1, r0 = 0, 127, t0
                else:
                    p0, p1, r0 = 122, 127, 762
                dst = o_d[r0:r0 + (p1 - p0), ig0:ig0 + IMGG, :]
                GP.dma_start(out=dst, in_=ot[p0:p1, :, 0:W])

        # ---- tail pass: output row 767, all 4 images on partitions 0..3 ----
        U7 = loads.tile([4, GW], F32, tag="a", name="u7")
        nc.sync.dma_start(out=U7[:, 0:W], in_=u_d[:, H - 1, :])
        SC.copy(U7[:, W:W + 2], U7[:, W - 1:W].to_broadcast([4, 2]))
        A7 = loads.tile([4, 2, 769], F32, tag="pu", name="a7")   # a' rows 767,768
        B7 = loads.tile([4, 2, 769], F32, tag="pu2", name="b7")
        nc.sync.dma_start(out=A7[:], in_=a_d[:, H:H + 2, 1:W + 2])
        nc.sync.dma_start(out=B7[:], in_=b_d[:, H:H + 2, 1:W + 2])
        D7 = scr.tile([4, 769], F32, tag="g1", name="d7t")
        V.tensor_sub(D7[:], U7[:, 1:GW], U7[:, 0:769])
        aa = scr.tile([4, 769], F32, tag="g2", name="aa7t")   # a'[767] + a'[768]
        V.tensor_add(aa[:], A7[:, 0, :], A7[:, 1, :])
        bb = scr.tile([4, 769], F32, tag="gp", name="bb7t")   # b'[768] - b'[767]
        V.tensor_sub(bb[:], B7[:, 1, :], B7[:, 0, :])
        sA = scr.tile([4, 769], F32, tag="gm", name="sa7t")   # s12[768]+s12[767]
        V.scalar_tensor_tensor(sA[:], aa[:], 2.0 * k4, D7[:], OP.mult, OP.mult)
        sB = scr.tile([4, 769], F32, tag="m", name="sb7t")    # s34[768]-s34[767]
        V.scalar_tensor_tensor(sB[:], bb[:], 2.0 * k4, D7[:], OP.mult, OP.mult)
        tX = scr.tile([4, W], F32, tag="t1", name="tx7t")
        V.tensor_sub(tX[:], sA[:, 1:769], sA[:, 0:W])
        tS = scr.tile([4, W], F32, tag="t2", name="ts7t")
        V.tensor_add(tS[:], sB[:, 1:769], sB[:, 0:W])
        tZ = scr.tile([4, W], F32, tag="s12", name="tz7t")
        V.tensor_add(tZ[:], tX[:], tS[:])
        o7 = scr.tile([4, W], F16, tag="s34", name="o77t")
        V.tensor_add(o7[:], tZ[:], U7[:, 0:W])
        GP.dma_start(out=o_d[H - 1, :, :], in_=o7[:])
    nc.finalize()
    return nc


def _smat(one_minus_2alpha):
    sh = np.zeros((128, 128), dtype=np.float32)
    for p in range(127):
        sh[p + 1, p] = 1.0   # sh[k, p] = 1 iff k = p+1  -> out[p] = in[p+1]
    ident = np.eye(128, dtype=np.float32)
    w1 = sh + ident
    w2 = sh - ident
    w2d = one_minus_2alpha * w2
    s = np.stack([w1, -w1, w2, -w2, w2d, -w2d])
    return s.astype(np.float16)


_cache = {}


def _get_nc(k4, k4m):
    key = (k4, k4m)
    if key not in _cache:
        _cache[key] = _build(k4, k4m)
    return _cache[key]


def kernel(u, a, b, c, grad_x1, grad_x2, grad_y1, grad_y2, alpha, tau):
    u = np.ascontiguousarray(np.asarray(u, dtype=np.float32))
    a = np.ascontiguousarray(np.asarray(a, dtype=np.float32))
    b = np.ascontiguousarray(np.asarray(b, dtype=np.float32))
    c = np.ascontiguousarray(np.asarray(c, dtype=np.float32))
    alpha_f = float(np.asarray(alpha))
    tau_f = float(np.asarray(tau))
    k4 = tau_f / 8.0
    k4m = tau_f * (1.0 - 2.0 * alpha_f) / 8.0

    nc = _get_nc(k4, k4m)
    smat = _smat(1.0 - 2.0 * alpha_f)

    bpc = B // NCORES  # batches per core
    in_maps = []
    for k in range(NCORES):
        sl = slice(bpc * k, bpc * (k + 1))
        in_maps.append({
            "u": np.ascontiguousarray(u[sl].reshape(NIMG, H, W)),
            "a": np.ascontiguousarray(a[sl].reshape(NIMG, H + 2, W + 2)),
            "b": np.ascontiguousarray(b[sl].reshape(NIMG, H + 2, W + 2)),
            "c": np.ascontiguousarray(c[sl].reshape(NIMG, H + 2, W + 2)),
            "smat": smat,
        })

    res = run_bass_kernel_spmd(nc, in_maps, list(range(NCORES)))
    out = np.empty((B, C, H, W), dtype=np.float32)
    for k in range(NCORES):
        r = np.asarray(res.results[k]["out"])          # [H, NIMG, W] fp16
        out[bpc * k:bpc * (k + 1)] = (
            np.moveaxis(r, 0, 1).astype(np.float32).reshape(bpc, C, H, W))
    return out

